# revision 1
# baseline (speedup 1.0000x reference)
"""Two-layer GAT (8-head + 1-head) Trainium2 Bass kernel, 8-way node-sharded.

Strategy (per core c, owning row block I_c of R = N/8 nodes):
  * Scores live in TRANSPOSED layout [j (partition), i (free)] so the
    aggregation matmul out^T[o, i] = sum_j h[j, o] * P[j, i] contracts over
    the partition dim naturally (lhsT = h rows, rhs = P^T tile).
  * e[h, j, i] = f_dst[h, j] + f_src[h, i] comes from a rank-3 matmul
    (lhsT rows = [ones, f_dst_h0, f_dst_h1] at a 32-aligned partition base,
    rhs = packed f_src row + block-diag ones rows), two heads per PSUM tile.
  * exp(leakyrelu(e)) == max(exp(e), exp(0.2 e)).  The exp(e) branch is one
    ScalarE pass; the exp(0.2 e) branch is separable (exp(.2 f_dst[j]) *
    exp(.2 f_src[i])) and becomes a VectorE tensor_scalar multiply against
    precomputed factors.  Masking is an elementwise multiply with adj^T
    (binary, exact in bf16), split across VectorE and GpSimd.
  * Z_i = sum_j P[j, i] via matmuls with a head-selector stationary matrix
    (heads packed into one PSUM bank at 32-partition offsets).
  * f vectors fall out of the feature transform for free by running it with
    the folded vector w = W @ a as the STATIONARY operand and the x^T tile
    moving: the result lands pre-transposed at exactly the partitions the
    score matmuls need.
  * Layer-2 inputs h2 = h1 @ W2 (+ f_src2/f_dst2) are tiny ([N, 18]); each
    core computes its own block and an AllGather distributes them.
"""

import sys

sys.path.insert(0, "/opt/trn_rl_repo")

import numpy as np
import ml_dtypes

N = 4096
F_IN = 512
H1 = 8
D1 = 128
F1 = 1024          # H1 * D1
D2 = 16
NCORES = 8
R = N // NCORES    # rows (nodes) per core
NCH = N // 128     # j-chunks of 128
NFC = F_IN // 128  # f chunks
ALPHA = 0.2

_BUILD_CACHE = {}


def _build_nc():
    import concourse.bacc as bacc
    import concourse.tile as tile
    import concourse.mybir as mybir

    FP32 = mybir.dt.float32
    BF16 = mybir.dt.bfloat16
    AF = mybir.ActivationFunctionType
    OP = mybir.AluOpType
    AX = mybir.AxisListType

    nc = bacc.Bacc(num_devices=NCORES)

    # ---- I/O -------------------------------------------------------------
    xT_d = nc.dram_tensor("xT", [F_IN, N], BF16, kind="ExternalInput")
    xTo_d = nc.dram_tensor("xTo", [F_IN, R], BF16, kind="ExternalInput")
    W1_d = nc.dram_tensor("W1f", [F_IN, F1], BF16, kind="ExternalInput")
    wsd1_d = nc.dram_tensor("wsd1", [F_IN, 128], BF16, kind="ExternalInput")
    wso1_d = nc.dram_tensor("wso1", [F_IN, 256], BF16, kind="ExternalInput")
    wdn_d = nc.dram_tensor("wdn", [F_IN, 8], BF16, kind="ExternalInput")
    adjT_d = nc.dram_tensor("adjT", [N, R], BF16, kind="ExternalInput")
    blk_d = nc.dram_tensor("blk", [2, 2 * R], BF16, kind="ExternalInput")
    onesrow_d = nc.dram_tensor("onesrow", [1, N], BF16, kind="ExternalInput")
    zsel_d = nc.dram_tensor("zsel", [128, 2 * 128], BF16, kind="ExternalInput")
    ones1_d = nc.dram_tensor("ones1", [128, 128], FP32, kind="ExternalInput")
    onesb_d = nc.dram_tensor("onesb", [128, 128], BF16, kind="ExternalInput")
    idb_d = nc.dram_tensor("idb", [128, 128], BF16, kind="ExternalInput")
    idf_d = nc.dram_tensor("idf", [128, 128], FP32, kind="ExternalInput")
    W2a_d = nc.dram_tensor("W2a", [F1, 18], BF16, kind="ExternalInput")
    out_d = nc.dram_tensor("out", [R, D2], FP32, kind="ExternalOutput")

    def dma_chunked(dst_tile, src_ap, inner, batch=False):
        # [C*128, inner] DRAM -> [128, C*inner] SBUF (chunk-major free dim)
        if batch:
            nc.sync.dma_start(
                dst_tile[:].rearrange("p (c o) -> p c o", o=inner),
                src_ap.rearrange("(c p) o -> p c o", p=128),
            )
            return
        nchunks = src_ap.shape[0] // 128
        for cc in range(nchunks):
            nc.sync.dma_start(
                dst_tile[:, cc * inner:(cc + 1) * inner],
                src_ap[cc * 128:(cc + 1) * 128, :],
            )

    with tile.TileContext(nc) as tc:
        with (
            tc.tile_pool(name="const", bufs=1) as cpool,
            tc.tile_pool(name="dram", bufs=1, space="DRAM") as dpool,
        ):
            # ---- resident SBUF tensors ----------------------------------
            W1_sb = cpool.tile([128, NFC * F1], BF16, tag="W1")
            dma_chunked(W1_sb, W1_d[:], F1)
            wsd1_sb = cpool.tile([128, NFC * 128], BF16, tag="wsd1")
            dma_chunked(wsd1_sb, wsd1_d[:], 128)
            wso1_sb = cpool.tile([128, NFC * 256], BF16, tag="wso1")
            dma_chunked(wso1_sb, wso1_d[:], 256)
            wdn_sb = cpool.tile([128, NFC * 8], BF16, tag="wdn")
            dma_chunked(wdn_sb, wdn_d[:], 8)
            adjT_sb = cpool.tile([128, NCH * R], BF16, tag="adjT")
            dma_chunked(adjT_sb, adjT_d[:], R)
            zsel_sb = cpool.tile([128, 2 * 128], BF16, tag="zsel")
            nc.sync.dma_start(zsel_sb[:], zsel_d[:])
            ones1_sb = cpool.tile([128, 128], FP32, tag="ones1")
            nc.sync.dma_start(ones1_sb[:], ones1_d[:])
            onesb_sb = cpool.tile([128, 128], BF16, tag="onesb")
            nc.sync.dma_start(onesb_sb[:], onesb_d[:])
            idb_sb = cpool.tile([128, 128], BF16, tag="idb")
            nc.sync.dma_start(idb_sb[:], idb_d[:])
            idf_sb = cpool.tile([128, 128], FP32, tag="idf")
            nc.sync.dma_start(idf_sb[:], idf_d[:])
            W2a_sb = cpool.tile([128, (F1 // 128) * 18], BF16, tag="W2a")
            dma_chunked(W2a_sb, W2a_d[:], 18)

            h_sb = cpool.tile([128, NCH * F1], BF16, tag="h")
            fdT = cpool.tile([128, N], BF16, tag="fdT")
            srcpat = cpool.tile([128, 2 * R], BF16, tag="srcpat")
            vd_sb = cpool.tile([128, NCH * 8], FP32, tag="vd")
            h1T = cpool.tile([128, H1 * R], BF16, tag="h1T")
            h2all_sb = cpool.tile([128, NCH * 18], BF16, tag="h2all")
            f2T = cpool.tile([2, N], BF16, tag="f2T")
            srcpat2 = cpool.tile([2, R], BF16, tag="srcpat2")
            fs2row = cpool.tile([1, R], BF16, tag="fs2row")
            vd2_sb = cpool.tile([128, NCH], FP32, tag="vd2")
            vbc2 = cpool.tile([128, R], BF16, tag="vbc2")

            h2loc = dpool.tile([R, 18], BF16, tag="h2loc")
            h2all_d = dpool.tile([N, 18], BF16, tag="h2all", addr_space="Shared")

            # srcpat block-diag rows; fdT ones rows (DMA: any partition base)
            for p in range(4):
                nc.sync.dma_start(srcpat[32 * p + 1:32 * p + 3, :], blk_d[:])
                nc.sync.dma_start(fdT[32 * p:32 * p + 1, :], onesrow_d[:])

            # =============================================================
            # Phase 1: h = x @ W1 (all nodes, replicated); f_dst^T rows via
            # stationary-w matmuls; v_dst = exp(.2 f_dst) columns; own-block
            # f_src^T rows.
            # =============================================================
            with (
                tc.tile_pool(name="xt", bufs=3) as xpool,
                tc.tile_pool(name="ph", bufs=2, space="PSUM") as php,
                tc.tile_pool(name="pf", bufs=2, space="PSUM") as pfp,
            ):
                # own block f_src^T rows, written at partitions 32p directly
                for jt2 in range(R // 128):
                    xo = xpool.tile([128, NFC * 128], BF16, tag="xt")
                    dma_chunked(xo, xTo_d[:, jt2 * 128:(jt2 + 1) * 128], 128, batch=True)
                    for k in range(2):
                        pfo = pfp.tile([128, 128], FP32, tag="pft")
                        for fc in range(NFC):
                            nc.tensor.matmul(
                                pfo[:],
                                wso1_sb[:, fc * 256 + 128 * k:fc * 256 + 128 * (k + 1)],
                                xo[:, fc * 128:(fc + 1) * 128],
                                start=fc == 0, stop=fc == NFC - 1,
                            )
                        for p in range(4):
                            nc.vector.tensor_copy(
                                srcpat[32 * p:32 * p + 1,
                                       k * R + jt2 * 128:k * R + (jt2 + 1) * 128],
                                pfo[32 * p:32 * p + 1, :],
                            )
                for jt in range(NCH):
                    xt = xpool.tile([128, NFC * 128], BF16, tag="xt")
                    dma_chunked(xt, xT_d[:, jt * 128:(jt + 1) * 128], 128, batch=True)
                    ph = php.tile([128, F1], FP32, tag="ph")
                    pft = pfp.tile([128, 128], FP32, tag="pft")
                    pfn = pfp.tile([128, 8], FP32, tag="pfn")
                    for fc in range(NFC):
                        lhs = xt[:, fc * 128:(fc + 1) * 128]
                        st = fc == 0
                        sp = fc == NFC - 1
                        nc.tensor.matmul(
                            ph[:, 0:512], lhs,
                            W1_sb[:, fc * F1:fc * F1 + 512], start=st, stop=sp,
                        )
                        nc.tensor.matmul(
                            ph[:, 512:F1], lhs,
                            W1_sb[:, fc * F1 + 512:(fc + 1) * F1], start=st, stop=sp,
                        )
                        nc.tensor.matmul(
                            pfn[:], lhs, wdn_sb[:, fc * 8:(fc + 1) * 8],
                            start=st, stop=sp,
                        )
                        nc.tensor.matmul(
                            pft[:], wsd1_sb[:, fc * 128:(fc + 1) * 128], lhs,
                            start=st, stop=sp,
                        )
                    # evacuate h (split DVE / ACT), f_dst^T, v_dst columns
                    nc.vector.tensor_copy(
                        h_sb[:, jt * F1:jt * F1 + 512], ph[:, 0:512]
                    )
                    nc.scalar.activation(
                        h_sb[:, jt * F1 + 512:(jt + 1) * F1], ph[:, 512:F1], AF.Copy
                    )
                    nc.vector.tensor_copy(
                        fdT[:, jt * 128:(jt + 1) * 128], pft[:]
                    )
                    nc.scalar.activation(
                        vd_sb[:, jt * 8:(jt + 1) * 8], pfn[:], AF.Exp, scale=ALPHA
                    )

            # =============================================================
            # Phase 2: layer-1 attention, 2 heads (one pair) per pass
            # =============================================================
            with (
                tc.tile_pool(name="acc", bufs=1, space="PSUM") as acc,
                tc.tile_pool(name="pe", bufs=2, space="PSUM") as epool,
                tc.tile_pool(name="zb", bufs=1, space="PSUM") as zbp,
                tc.tile_pool(name="sc", bufs=2) as spool,
                tc.tile_pool(name="pt", bufs=3) as ptpool,
                tc.tile_pool(name="vb", bufs=2) as vbpool,
                tc.tile_pool(name="nrm", bufs=1) as npool,
            ):
                for hp in range(4):
                    p = hp  # pair index; heads (2p, 2p+1)
                    # v_src broadcast tile for this pair
                    vsr = vbpool.tile([128, 2 * R], BF16, tag="vsr")
                    nc.scalar.activation(
                        vsr[32 * p:32 * p + 1, :], srcpat[32 * p:32 * p + 1, :],
                        AF.Exp, scale=ALPHA,
                    )
                    vbc = vbpool.tile([128, 2 * R], BF16, tag="vbc")
                    for k in range(2):
                        pvb = zbp.tile([128, R], FP32, tag="zb")
                        nc.tensor.matmul(
                            pvb[:],
                            onesb_sb[32 * p:32 * p + 1, :],
                            vsr[32 * p:32 * p + 1, k * R:(k + 1) * R],
                            start=True, stop=True, tile_position=(32 * p, 0),
                        )
                        nc.vector.tensor_copy(vbc[:, k * R:(k + 1) * R], pvb[:])

                    po = [
                        acc.tile([128, R], FP32, tag=f"o{k}", name=f"po{k}")
                        for k in range(2)
                    ]
                    pz = acc.tile([128, R], FP32, tag="z")
                    for c in range(NCH):
                        pe = epool.tile([128, 2 * R], FP32, tag="pe")
                        lhs_e = fdT[32 * p:32 * p + 3, c * 128:(c + 1) * 128]
                        nc.tensor.matmul(
                            pe[:, 0:R], lhs_e, srcpat[32 * p:32 * p + 3, 0:R],
                            start=True, stop=True, tile_position=(32 * p, 0),
                        )
                        nc.tensor.matmul(
                            pe[:, R:2 * R], lhs_e,
                            srcpat[32 * p:32 * p + 3, R:2 * R],
                            start=True, stop=True, tile_position=(32 * p, 0),
                        )
                        t1 = spool.tile([128, 2 * R], BF16, tag="t1")
                        nc.scalar.activation(t1[:], pe[:], AF.Exp)
                        vt = spool.tile([128, 2 * R], BF16, tag="vt")
                        for k in range(2):
                            nc.vector.tensor_scalar_mul(
                                vt[:, k * R:(k + 1) * R],
                                vbc[:, k * R:(k + 1) * R],
                                vd_sb[:, c * 8 + 2 * p + k:c * 8 + 2 * p + k + 1],
                            )
                        m = spool.tile([128, 2 * R], BF16, tag="m")
                        nc.vector.tensor_max(m[:], t1[:], vt[:])
                        pt = ptpool.tile([128, 2 * R], BF16, tag="pt")
                        nc.vector.tensor_mul(
                            pt[:, 0:R], m[:, 0:R], adjT_sb[:, c * R:(c + 1) * R]
                        )
                        nc.gpsimd.tensor_mul(
                            pt[:, R:2 * R], m[:, R:2 * R],
                            adjT_sb[:, c * R:(c + 1) * R],
                        )
                        for k in range(2):
                            g = 2 * p + k
                            nc.tensor.matmul(
                                po[k][:],
                                h_sb[:, c * F1 + g * D1:c * F1 + (g + 1) * D1],
                                pt[:, k * R:(k + 1) * R],
                                start=c == 0, stop=c == NCH - 1,
                            )
                            nc.tensor.matmul(
                                pz[:],
                                zsel_sb[:, k * 128:(k + 1) * 128],
                                pt[:, k * R:(k + 1) * R],
                                start=(c == 0 and k == 0),
                                stop=(c == NCH - 1 and k == 1),
                            )
                    # normalize + ELU -> h1^T (bf16)
                    zinv = npool.tile([128, R], FP32, tag="zinv")
                    nc.vector.reciprocal(zinv[:], pz[:])
                    for k in range(2):
                        g = 2 * p + k
                        zb_ps = zbp.tile([128, R], FP32, tag="zb")
                        nc.tensor.matmul(
                            zb_ps[:],
                            ones1_sb[32 * k:32 * k + 1, :],
                            zinv[32 * k:32 * k + 1, :],
                            start=True, stop=True,
                            tile_position=(32 * k, 0),
                        )
                        zb_sb = npool.tile([128, R], FP32, tag="zbs")
                        nc.vector.tensor_copy(zb_sb[:], zb_ps[:])
                        pre = npool.tile([128, R], FP32, tag="pre")
                        nc.vector.tensor_mul(pre[:], po[k][:], zb_sb[:])
                        r = npool.tile([128, R], FP32, tag="r")
                        nc.scalar.activation(r[:], pre[:], AF.Relu, scale=-1.0)
                        t = npool.tile([128, R], FP32, tag="t")
                        nc.scalar.activation(t[:], r[:], AF.Exp, scale=-1.0)
                        u = npool.tile([128, R], FP32, tag="u")
                        nc.vector.tensor_scalar_add(u[:], t[:], -1.0)
                        nc.vector.tensor_max(
                            h1T[:, g * R:(g + 1) * R], pre[:], u[:]
                        )

            # =============================================================
            # Phase 3: layer-2 transform + AllGather of [h2 | f_src2 | f_dst2]
            # =============================================================
            with (
                tc.tile_pool(name="p2", bufs=2, space="PSUM") as p2p,
                tc.tile_pool(name="p2t", bufs=2, space="PSUM") as p2tp,
                tc.tile_pool(name="h2s", bufs=2) as h2p,
            ):
                for jt2 in range(R // 128):
                    ph2 = p2p.tile([128, 18], FP32, tag="ph2")
                    for fc2 in range(F1 // 128):
                        nc.tensor.matmul(
                            ph2[:],
                            h1T[:, fc2 * R + jt2 * 128:fc2 * R + (jt2 + 1) * 128],
                            W2a_sb[:, fc2 * 18:(fc2 + 1) * 18],
                            start=fc2 == 0, stop=fc2 == F1 // 128 - 1,
                        )
                    h2t = h2p.tile([128, 18], BF16, tag="h2t")
                    nc.vector.tensor_copy(h2t[:], ph2[:])
                    nc.sync.dma_start(
                        h2loc[jt2 * 128:(jt2 + 1) * 128, :], h2t[:]
                    )
                    # f_src2 row: transpose cols 16:18, stage row 0, DMA to
                    # srcpat2 row 1 (engine writes must be 32-part aligned)
                    ps2 = p2tp.tile([2, 128], BF16, tag="ps2")
                    nc.tensor.transpose(ps2[:], h2t[:, 16:18], idb_sb[:])
                    nc.vector.tensor_copy(
                        fs2row[0:1, jt2 * 128:(jt2 + 1) * 128], ps2[0:1, :]
                    )
                nc.vector.memset(srcpat2[0:1, :], 1.0)
                nc.sync.dma_start(srcpat2[1:2, :], fs2row[:])

                nc.gpsimd.collective_compute(
                    "AllGather",
                    OP.bypass,
                    replica_groups=[list(range(NCORES))],
                    ins=[h2loc[:].opt()],
                    outs=[h2all_d[:].opt()],
                )
                dma_chunked(h2all_sb, h2all_d[:], 18, batch=True)
                # global f_dst2 row via single-column PE transposes
                for c in range(NCH):
                    pcol = p2tp.tile([1, 128], BF16, tag="pcol")
                    nc.tensor.transpose(
                        pcol[:], h2all_sb[:, c * 18 + 17:c * 18 + 18], idb_sb[:]
                    )
                    nc.vector.tensor_copy(
                        f2T[0:1, c * 128:(c + 1) * 128], pcol[:]
                    )
                nc.sync.dma_start(f2T[1:2, :], onesrow_d[:])
                # layer-2 exp(.2 f) factors
                nc.scalar.activation(
                    vd2_sb[:, 0:NCH], h2all_sb[:, 17:NCH * 18:18],
                    AF.Exp, scale=ALPHA,
                )
                vsr2 = h2p.tile([1, R], BF16, tag="vsr2")
                nc.scalar.activation(vsr2[:], fs2row[:], AF.Exp, scale=ALPHA)
                pvb2 = p2tp.tile([128, R], FP32, tag="pvb2")
                nc.tensor.matmul(
                    pvb2[:], onesb_sb[0:1, :], vsr2[0:1, :],
                    start=True, stop=True,
                )
                nc.vector.tensor_copy(vbc2[:], pvb2[:])

            # =============================================================
            # Phase 4: layer-2 attention + ELU + log_softmax
            # =============================================================
            with (
                tc.tile_pool(name="acc2", bufs=1, space="PSUM") as acc2,
                tc.tile_pool(name="pe2", bufs=2, space="PSUM") as e2pool,
                tc.tile_pool(name="sc2", bufs=2) as spool2,
                tc.tile_pool(name="fin", bufs=2) as fpool,
                tc.tile_pool(name="pfin", bufs=2, space="PSUM") as pfp2,
            ):
                po2 = acc2.tile([16, R], FP32, tag="o2")
                pz2 = acc2.tile([128, R], FP32, tag="z2")
                for c in range(NCH):
                    pe2 = e2pool.tile([128, R], FP32, tag="pe2")
                    nc.tensor.matmul(
                        pe2[:], f2T[:, c * 128:(c + 1) * 128], srcpat2[:],
                        start=True, stop=True,
                    )
                    t1 = spool2.tile([128, R], BF16, tag="t1b")
                    nc.scalar.activation(t1[:], pe2[:], AF.Exp)
                    t2 = spool2.tile([128, R], BF16, tag="t2b")
                    nc.vector.tensor_scalar_mul(
                        t2[:], vbc2[:], vd2_sb[:, c:c + 1]
                    )
                    m2 = spool2.tile([128, R], BF16, tag="m2")
                    nc.vector.tensor_max(m2[:], t1[:], t2[:])
                    pt2 = spool2.tile([128, R], BF16, tag="pt2")
                    nc.vector.tensor_mul(
                        pt2[:], m2[:], adjT_sb[:, c * R:(c + 1) * R]
                    )
                    nc.tensor.matmul(
                        po2[:], h2all_sb[:, c * 18:c * 18 + 16], pt2[:],
                        start=c == 0, stop=c == NCH - 1,
                    )
                    nc.tensor.matmul(
                        pz2[:], onesb_sb[:], pt2[:],
                        start=c == 0, stop=c == NCH - 1,
                    )
                zinv2 = fpool.tile([128, R], FP32, tag="zinv2")
                nc.vector.reciprocal(zinv2[:], pz2[:])
                pre2 = fpool.tile([16, R], FP32, tag="pre2")
                nc.vector.tensor_mul(pre2[:], po2[:], zinv2[0:16, :])
                r2 = fpool.tile([16, R], FP32, tag="r2")
                nc.scalar.activation(r2[:], pre2[:], AF.Relu, scale=-1.0)
                t2e = fpool.tile([16, R], FP32, tag="t2e")
                nc.scalar.activation(t2e[:], r2[:], AF.Exp, scale=-1.0)
                u2 = fpool.tile([16, R], FP32, tag="u2")
                nc.vector.tensor_scalar_add(u2[:], t2e[:], -1.0)
                elu2 = fpool.tile([16, R], FP32, tag="elu2")
                nc.vector.tensor_max(elu2[:], pre2[:], u2[:])
                # transpose to natural [i, o2] then log_softmax over free dim
                for it in range(R // 128):
                    pn = pfp2.tile([128, 16], FP32, tag="pn")
                    nc.tensor.transpose(
                        pn[:], elu2[:, it * 128:(it + 1) * 128],
                        idf_sb[0:16, 0:16],
                    )
                    nmx = fpool.tile([128, 1], FP32, tag="nmx")
                    nc.vector.tensor_reduce(
                        nmx[:], pn[:], AX.X, OP.max, negate=True
                    )
                    ex = fpool.tile([128, 16], FP32, tag="ex")
                    s = fpool.tile([128, 1], FP32, tag="s")
                    nc.scalar.activation(
                        ex[:], pn[:], AF.Exp, bias=nmx[:, 0:1], accum_out=s[:, 0:1]
                    )
                    lg = fpool.tile([128, 1], FP32, tag="lg")
                    nc.scalar.activation(lg[:], s[:], AF.Ln)
                    fin = fpool.tile([128, 16], FP32, tag="fin")
                    nc.vector.tensor_scalar(
                        fin[:], pn[:], nmx[:, 0:1], lg[:, 0:1],
                        op0=OP.add, op1=OP.subtract,
                    )
                    nc.sync.dma_start(out_d[it * 128:(it + 1) * 128, :], fin[:])

    nc.compile()
    return nc


def _get_nc():
    if "nc" not in _BUILD_CACHE:
        _BUILD_CACHE["nc"] = _build_nc()
    return _BUILD_CACHE["nc"]


def _prep_inputs(x, adj, W1, a_src1, a_dst1, W2, a_src2, a_dst2):
    bf16 = ml_dtypes.bfloat16
    f32 = np.float32
    x = np.asarray(x, f32)
    adj = np.asarray(adj, f32)
    W1 = np.asarray(W1, f32)
    W2 = np.asarray(W2, f32)
    a_src1 = np.asarray(a_src1, f32)
    a_dst1 = np.asarray(a_dst1, f32)
    a_src2 = np.asarray(a_src2, f32)
    a_dst2 = np.asarray(a_dst2, f32)

    W1f = np.ascontiguousarray(W1.reshape(F_IN, F1))
    # folded score vectors: f_src[h] = x @ (W1[:,h,:] @ a_src1[h])
    wsrc = np.stack([W1[:, h, :] @ a_src1[h] for h in range(H1)], axis=1)
    wdst = np.stack([W1[:, h, :] @ a_dst1[h] for h in range(H1)], axis=1)
    # pair p lives at partitions 32p..32p+2: [ones, fd_2p, fd_2p+1]
    wsd1 = np.zeros((F_IN, 128), f32)
    for p in range(4):
        wsd1[:, 32 * p + 1] = wdst[:, 2 * p]
        wsd1[:, 32 * p + 2] = wdst[:, 2 * p + 1]
    # f_src columns placed so the transform emits rows at partition 32p
    wso1 = np.zeros((F_IN, 256), f32)
    for k in range(2):
        for p in range(4):
            wso1[:, 128 * k + 32 * p] = wsrc[:, 2 * p + k]
    W2f = np.ascontiguousarray(W2.reshape(F1, D2))
    W2a = np.zeros((F1, 18), f32)
    W2a[:, :D2] = W2f
    W2a[:, 16] = W2f @ a_src2[0]
    W2a[:, 17] = W2f @ a_dst2[0]

    xT = np.ascontiguousarray(x.T)
    blk = np.zeros((2, 2 * R), f32)
    blk[0, :R] = 1.0
    blk[1, R:] = 1.0
    zsel = np.zeros((128, 2 * 128), f32)
    for lh in range(2):
        zsel[:, 128 * lh + 32 * lh:128 * lh + 32 * lh + 32] = 1.0
    ident = np.eye(128, dtype=f32)

    shared = {
        "xT": xT.astype(bf16),
        "W1f": W1f.astype(bf16),
        "wsd1": wsd1.astype(bf16),
        "wso1": wso1.astype(bf16),
        "wdn": wdst.astype(bf16),
        "blk": blk.astype(bf16),
        "onesrow": np.ones((1, N), bf16),
        "zsel": zsel.astype(bf16),
        "ones1": np.ones((128, 128), f32),
        "onesb": np.ones((128, 128), bf16),
        "idb": ident.astype(bf16),
        "idf": ident,
        "W2a": W2a.astype(bf16),
    }
    in_maps = []
    for c in range(NCORES):
        blkslice = slice(c * R, (c + 1) * R)
        m = dict(shared)
        m["adjT"] = np.ascontiguousarray(adj[blkslice, :].T).astype(bf16)
        m["xTo"] = np.ascontiguousarray(x[blkslice, :].T).astype(bf16)
        in_maps.append(m)
    return in_maps


def kernel(x, adj, W1, a_src1, a_dst1, W2, a_src2, a_dst2, _trace=False):
    from concourse.bass_utils import run_bass_kernel_spmd

    nc = _get_nc()
    in_maps = _prep_inputs(x, adj, W1, a_src1, a_dst1, W2, a_src2, a_dst2)
    res = run_bass_kernel_spmd(nc, in_maps, list(range(NCORES)), trace=_trace)
    out = np.concatenate(
        [np.asarray(res.results[c]["out"]) for c in range(NCORES)], axis=0
    )
    kernel.last_results = res
    return out.astype(np.float32)



# revision 2
# speedup vs baseline: 1.6536x; 1.6536x over previous
"""Two-layer GAT (8-head + 1-head) Trainium2 Bass kernel, 8-way node-sharded.

Strategy (per core c, owning row block I_c of R = N/8 nodes), layer 1:
  * Softmax over neighbors j is invariant to per-row (per-i) scaling, so
    P[j, i] ~ adjT[j, i] * max(a_i * exp(fd_j), exp(alpha * fd_j)) with
    a_i = exp((1-alpha) * fs_i), using exp(leakyrelu(z)) = max(exp(z),
    exp(alpha z)) and dropping the exp(alpha fs_i) row factor.  exp is
    evaluated only on N-sized score vectors (fs = x @ (W1 @ a_src),
    fd = x @ (W1 @ a_dst)); the N x R attention field needs just TWO
    fused DVE ops per tile: u = max(abc * efd_j, vd_j) (tensor_scalar
    with two per-partition scalars) and p = u * adjT (mask multiply).
  * Scores live in TRANSPOSED layout [j (partition), i (free)] so both
    the aggregation out^T[o, i] = sum_j h[j, o] P[j, i] and the softmax
    denominator Z contract over the partition dim.  Z rides an all-ones
    stationary matmul, landing broadcast across all 128 partitions so
    normalization needs no further broadcast.
  * h = x @ W1 for all nodes is computed locally (replicated), with the
    fd columns falling out of the same pass via a folded [512, 8]
    matmul; exp(fd)/exp(alpha fd) are tiny per-chunk ACT ops.
  * Layer-2 inputs h2 = h1 @ W2 (+ f_src2/f_dst2 via folded W2 columns)
    are tiny ([N, 18]); each core computes its own block and an
    AllGather distributes them; layer 2 repeats the same scheme with a
    single head.
"""

import sys

sys.path.insert(0, "/opt/trn_rl_repo")

import numpy as np
import ml_dtypes

N = 4096
F_IN = 512
H1 = 8
D1 = 128
F1 = 1024          # H1 * D1
D2 = 16
NCORES = 8
R = N // NCORES    # rows (nodes) per core
NCH = N // 128     # j-chunks of 128
NFC = F_IN // 128  # f chunks
ALPHA = 0.2

_BUILD_CACHE = {}


def _build_nc():
    import concourse.bacc as bacc
    import concourse.tile as tile
    import concourse.mybir as mybir

    FP32 = mybir.dt.float32
    BF16 = mybir.dt.bfloat16
    AF = mybir.ActivationFunctionType
    OP = mybir.AluOpType
    AX = mybir.AxisListType

    nc = bacc.Bacc(num_devices=NCORES)

    # ---- I/O -------------------------------------------------------------
    xT_d = nc.dram_tensor("xT", [F_IN, N], BF16, kind="ExternalInput")
    xTo_d = nc.dram_tensor("xTo", [F_IN, R], BF16, kind="ExternalInput")
    W1_d = nc.dram_tensor("W1f", [F_IN, F1], BF16, kind="ExternalInput")
    wdn_d = nc.dram_tensor("wdn", [F_IN, 8], BF16, kind="ExternalInput")
    wsn_d = nc.dram_tensor("wsn", [F_IN, 8], BF16, kind="ExternalInput")
    adjT_d = nc.dram_tensor("adjT", [N, R], BF16, kind="ExternalInput")
    onesb_d = nc.dram_tensor("onesb", [128, 128], BF16, kind="ExternalInput")
    idb_d = nc.dram_tensor("idb", [128, 128], BF16, kind="ExternalInput")
    idf_d = nc.dram_tensor("idf16", [16, 16], FP32, kind="ExternalInput")
    W2a_d = nc.dram_tensor("W2a", [F1, 18], BF16, kind="ExternalInput")
    out_d = nc.dram_tensor("out", [R, D2], FP32, kind="ExternalOutput")

    with tile.TileContext(nc) as tc:
        with (
            tc.tile_pool(name="const", bufs=1) as cpool,
            tc.tile_pool(name="dram", bufs=1, space="DRAM") as dpool,
        ):
            # ---- resident SBUF tensors ----------------------------------
            W1_sb = cpool.tile([128, NFC * F1], BF16, tag="W1")
            for fc in range(NFC):
                nc.sync.dma_start(
                    W1_sb[:, fc * F1:(fc + 1) * F1],
                    W1_d[fc * 128:(fc + 1) * 128, :],
                )
            wdn_sb = cpool.tile([128, NFC * 8], BF16, tag="wdn")
            wsn_sb = cpool.tile([128, NFC * 8], BF16, tag="wsn")
            xo_sb = cpool.tile([128, NFC * R], BF16, tag="xo")
            for fc in range(NFC):
                nc.sync.dma_start(
                    wdn_sb[:, fc * 8:(fc + 1) * 8],
                    wdn_d[fc * 128:(fc + 1) * 128, :],
                )
                nc.sync.dma_start(
                    wsn_sb[:, fc * 8:(fc + 1) * 8],
                    wsn_d[fc * 128:(fc + 1) * 128, :],
                )
                nc.sync.dma_start(
                    xo_sb[:, fc * R:(fc + 1) * R],
                    xTo_d[fc * 128:(fc + 1) * 128, :],
                )
            onesb_sb = cpool.tile([128, 128], BF16, tag="onesb")
            nc.sync.dma_start(onesb_sb[:], onesb_d[:])
            idb_sb = cpool.tile([128, 128], BF16, tag="idb")
            nc.sync.dma_start(idb_sb[:], idb_d[:])
            idf_sb = cpool.tile([16, 16], FP32, tag="idf")
            nc.sync.dma_start(idf_sb[:], idf_d[:])
            W2a_sb = cpool.tile([128, (F1 // 128) * 18], BF16, tag="W2a")
            for g in range(F1 // 128):
                nc.sync.dma_start(
                    W2a_sb[:, g * 18:(g + 1) * 18],
                    W2a_d[g * 128:(g + 1) * 128, :],
                )
            adjT_sb = cpool.tile([128, NCH * R], BF16, tag="adjT")

            h_sb = cpool.tile([128, NCH * F1], BF16, tag="h")
            abc = cpool.tile([128, H1 * R], BF16, tag="abc")
            a8x = cpool.tile([128, 2 * R], BF16, tag="a8x")
            efd_sb = cpool.tile([128, NCH * 8], FP32, tag="efd")
            vd_sb = cpool.tile([128, NCH * 8], FP32, tag="vd")
            h1T = cpool.tile([128, H1 * R], BF16, tag="h1T")
            h2all_sb = cpool.tile([128, NCH * 18], BF16, tag="h2all")
            fs2row = cpool.tile([1, R], FP32, tag="fs2row")
            abc2 = cpool.tile([128, R], BF16, tag="abc2")
            efd2_sb = cpool.tile([128, NCH], FP32, tag="efd2")
            vd2_sb = cpool.tile([128, NCH], FP32, tag="vd2")

            h2loc = dpool.tile([R, 18], BF16, tag="h2loc")
            h2all_d = dpool.tile([N, 18], BF16, tag="h2all", addr_space="Shared")

            # =============================================================
            # Phases A+B share the streamed-in xT (scoped: freed after B)
            # =============================================================
            with tc.tile_pool(name="xres", bufs=1) as xrp:
                x_sb = xrp.tile([128, NFC * N], BF16, tag="x")
                # quarter-column DMAs, q-major so early jt chunks land first
                for q in range(4):
                    for fc in range(NFC):
                        nc.sync.dma_start(
                            x_sb[:, fc * N + q * 1024:fc * N + (q + 1) * 1024],
                            xT_d[fc * 128:(fc + 1) * 128,
                                 q * 1024:(q + 1) * 1024],
                        )
                # adjT after x so phase B isn't starved
                for c in range(NCH):
                    nc.sync.dma_start(
                        adjT_sb[:, c * R:(c + 1) * R],
                        adjT_d[c * 128:(c + 1) * 128, :],
                    )

                # ---- Phase A: own-block f_src -> a broadcast tiles ------
                with (
                    tc.tile_pool(name="pfs", bufs=1, space="PSUM") as pfsp,
                    tc.tile_pool(name="pab", bufs=2, space="PSUM") as pabp,
                    tc.tile_pool(name="a8p", bufs=1) as a8p,
                ):
                    fsT8 = pfsp.tile([8, R], FP32, tag="fs8")
                    for fc in range(NFC):
                        nc.tensor.matmul(
                            fsT8[:],
                            wsn_sb[:, fc * 8:(fc + 1) * 8],
                            xo_sb[:, fc * R:(fc + 1) * R],
                            start=fc == 0, stop=fc == NFC - 1,
                        )
                    a8 = a8p.tile([8, R], BF16, tag="a8")
                    nc.scalar.activation(a8[:], fsT8[:], AF.Exp, scale=1.0 - ALPHA)
                    for g in range(H1):
                        q, hf = g % 4, g // 4
                        nc.sync.dma_start(
                            a8x[32 * q:32 * q + 1, hf * R:(hf + 1) * R],
                            a8[g:g + 1, :],
                        )
                    for g in range(H1):
                        q, hf = g % 4, g // 4
                        pb = pabp.tile([128, R], FP32, tag="pab")
                        nc.tensor.matmul(
                            pb[:],
                            onesb_sb[32 * q:32 * q + 1, :],
                            a8x[32 * q:32 * q + 1, hf * R:(hf + 1) * R],
                            start=True, stop=True, tile_position=(32 * q, 0),
                        )
                        nc.scalar.activation(
                            abc[:, g * R:(g + 1) * R], pb[:], AF.Copy
                        )

                # ---- Phase B: h = x @ W1 (all nodes) + fd columns -------
                with (
                    tc.tile_pool(name="ph", bufs=2, space="PSUM") as php,
                    tc.tile_pool(name="pf", bufs=2, space="PSUM") as pfp,
                ):
                    for jt in range(NCH):
                        ph = php.tile([128, F1], FP32, tag="ph")
                        pfn = pfp.tile([128, 8], FP32, tag="pfn")
                        for fc in range(NFC):
                            lhs = x_sb[:, fc * N + jt * 128:fc * N + (jt + 1) * 128]
                            st, sp = fc == 0, fc == NFC - 1
                            nc.tensor.matmul(
                                ph[:, 0:512], lhs,
                                W1_sb[:, fc * F1:fc * F1 + 512],
                                start=st, stop=sp,
                            )
                            nc.tensor.matmul(
                                ph[:, 512:F1], lhs,
                                W1_sb[:, fc * F1 + 512:(fc + 1) * F1],
                                start=st, stop=sp,
                            )
                            nc.tensor.matmul(
                                pfn[:], lhs, wdn_sb[:, fc * 8:(fc + 1) * 8],
                                start=st, stop=sp,
                            )
                        nc.scalar.activation(
                            h_sb[:, jt * F1:jt * F1 + 512], ph[:, 0:512], AF.Copy
                        )
                        nc.vector.tensor_copy(
                            h_sb[:, jt * F1 + 512:(jt + 1) * F1], ph[:, 512:F1]
                        )
                        nc.scalar.activation(
                            efd_sb[:, jt * 8:(jt + 1) * 8], pfn[:], AF.Exp
                        )
                        nc.scalar.activation(
                            vd_sb[:, jt * 8:(jt + 1) * 8], pfn[:], AF.Exp,
                            scale=ALPHA,
                        )

            # =============================================================
            # Phase C: layer-1 attention, 2 heads (one pair) per pass
            # =============================================================
            with (
                tc.tile_pool(name="acc", bufs=2, space="PSUM") as acc,
                tc.tile_pool(name="sc", bufs=3) as spool,
                tc.tile_pool(name="nrm", bufs=2) as npool,
            ):
                for p in range(4):
                    g0, g1 = 2 * p, 2 * p + 1
                    po0 = acc.tile([128, R], FP32, tag="po0", name="po0")
                    po1 = acc.tile([128, R], FP32, tag="po1", name="po1")
                    pz = acc.tile([128, 2 * R], FP32, tag="pz", name="pz")
                    for c in range(NCH):
                        up = spool.tile([128, 2 * R], BF16, tag="up")
                        pp = spool.tile([128, 2 * R], BF16, tag="pp")
                        for k, g in ((0, g0), (1, g1)):
                            nc.vector.tensor_scalar(
                                up[:, k * R:(k + 1) * R],
                                abc[:, g * R:(g + 1) * R],
                                efd_sb[:, c * 8 + g:c * 8 + g + 1],
                                vd_sb[:, c * 8 + g:c * 8 + g + 1],
                                op0=OP.mult, op1=OP.max,
                            )
                            nc.vector.tensor_mul(
                                pp[:, k * R:(k + 1) * R],
                                up[:, k * R:(k + 1) * R],
                                adjT_sb[:, c * R:(c + 1) * R],
                            )
                        nc.tensor.matmul(
                            po0[:],
                            h_sb[:, c * F1 + g0 * D1:c * F1 + (g0 + 1) * D1],
                            pp[:, 0:R],
                            start=c == 0, stop=c == NCH - 1,
                        )
                        nc.tensor.matmul(
                            pz[:, 0:R], onesb_sb[:], pp[:, 0:R],
                            start=c == 0, stop=c == NCH - 1,
                        )
                        nc.tensor.matmul(
                            po1[:],
                            h_sb[:, c * F1 + g1 * D1:c * F1 + (g1 + 1) * D1],
                            pp[:, R:2 * R],
                            start=c == 0, stop=c == NCH - 1,
                        )
                        nc.tensor.matmul(
                            pz[:, R:2 * R], onesb_sb[:], pp[:, R:2 * R],
                            start=c == 0, stop=c == NCH - 1,
                        )
                    # normalize + ELU -> h1^T (bf16)
                    zr = npool.tile([128, 2 * R], FP32, tag="zr")
                    nc.vector.reciprocal_approx_fast(zr[:], pz[:])
                    for k, g, po in ((0, g0, po0), (1, g1, po1)):
                        pre = npool.tile([128, R], FP32, tag="pre")
                        nc.vector.tensor_mul(
                            pre[:], po[:], zr[:, k * R:(k + 1) * R]
                        )
                        r = npool.tile([128, R], FP32, tag="r")
                        nc.scalar.activation(r[:], pre[:], AF.Relu, scale=-1.0)
                        t = npool.tile([128, R], FP32, tag="t")
                        nc.scalar.activation(t[:], r[:], AF.Exp, scale=-1.0)
                        nc.vector.scalar_tensor_tensor(
                            h1T[:, g * R:(g + 1) * R], t[:], -1.0, pre[:],
                            op0=OP.add, op1=OP.max,
                        )

            # =============================================================
            # Phase D: layer-2 transform + AllGather of [h2 | fs2 | fd2]
            # =============================================================
            with (
                tc.tile_pool(name="p2", bufs=2, space="PSUM") as p2p,
                tc.tile_pool(name="p2t", bufs=2, space="PSUM") as p2tp,
                tc.tile_pool(name="h2s", bufs=2) as h2p,
            ):
                for jt2 in range(R // 128):
                    ph2 = p2p.tile([128, 18], FP32, tag="ph2")
                    for g in range(H1):
                        nc.tensor.matmul(
                            ph2[:],
                            h1T[:, g * R + jt2 * 128:g * R + (jt2 + 1) * 128],
                            W2a_sb[:, g * 18:(g + 1) * 18],
                            start=g == 0, stop=g == H1 - 1,
                        )
                    h2t = h2p.tile([128, 18], BF16, tag="h2t")
                    nc.vector.tensor_copy(h2t[:], ph2[:])
                    nc.sync.dma_start(
                        h2loc[jt2 * 128:(jt2 + 1) * 128, :], h2t[:]
                    )
                    ps2 = p2tp.tile([1, 128], BF16, tag="ps2")
                    nc.tensor.transpose(ps2[:], h2t[:, 16:17], idb_sb[:])
                    nc.vector.tensor_copy(
                        fs2row[0:1, jt2 * 128:(jt2 + 1) * 128], ps2[:]
                    )

                nc.gpsimd.collective_compute(
                    "AllGather",
                    OP.bypass,
                    replica_groups=[list(range(NCORES))],
                    ins=[h2loc[:].opt()],
                    outs=[h2all_d[:].opt()],
                )
                nc.sync.dma_start(
                    h2all_sb[:].rearrange("p (c o) -> p c o", o=18),
                    h2all_d[:].rearrange("(c p) o -> p c o", p=128),
                )
                # layer-2 score factors
                nc.scalar.activation(
                    efd2_sb[:, 0:NCH], h2all_sb[:, 17:NCH * 18:18], AF.Exp
                )
                nc.scalar.activation(
                    vd2_sb[:, 0:NCH], h2all_sb[:, 17:NCH * 18:18], AF.Exp,
                    scale=ALPHA,
                )
                a2row = h2p.tile([1, R], BF16, tag="a2row")
                nc.scalar.activation(
                    a2row[:], fs2row[:], AF.Exp, scale=1.0 - ALPHA
                )
                pab2 = p2tp.tile([128, R], FP32, tag="pab2")
                nc.tensor.matmul(
                    pab2[:], onesb_sb[0:1, :], a2row[0:1, :],
                    start=True, stop=True,
                )
                nc.scalar.activation(abc2[:], pab2[:], AF.Copy)

            # =============================================================
            # Phase E: layer-2 attention + ELU + log_softmax
            # =============================================================
            with (
                tc.tile_pool(name="acc2", bufs=1, space="PSUM") as acc2,
                tc.tile_pool(name="sc2", bufs=3) as spool2,
                tc.tile_pool(name="fin", bufs=2) as fpool,
                tc.tile_pool(name="pfin", bufs=2, space="PSUM") as pfp2,
            ):
                po2 = acc2.tile([16, R], FP32, tag="o2")
                pz2 = acc2.tile([128, R], FP32, tag="z2")
                for c in range(NCH):
                    u2 = spool2.tile([128, R], BF16, tag="u2")
                    nc.vector.tensor_scalar(
                        u2[:], abc2[:],
                        efd2_sb[:, c:c + 1], vd2_sb[:, c:c + 1],
                        op0=OP.mult, op1=OP.max,
                    )
                    p2t = spool2.tile([128, R], BF16, tag="p2t")
                    nc.vector.tensor_mul(
                        p2t[:], u2[:], adjT_sb[:, c * R:(c + 1) * R]
                    )
                    nc.tensor.matmul(
                        po2[:], h2all_sb[:, c * 18:c * 18 + 16], p2t[:],
                        start=c == 0, stop=c == NCH - 1,
                    )
                    nc.tensor.matmul(
                        pz2[:], onesb_sb[:], p2t[:],
                        start=c == 0, stop=c == NCH - 1,
                    )
                zr2 = fpool.tile([16, R], FP32, tag="zr2")
                nc.vector.reciprocal_approx_fast(zr2[:], pz2[0:16, :])
                pre2 = fpool.tile([16, R], FP32, tag="pre2")
                nc.vector.tensor_mul(pre2[:], po2[:], zr2[:])
                r2 = fpool.tile([16, R], FP32, tag="r2")
                nc.scalar.activation(r2[:], pre2[:], AF.Relu, scale=-1.0)
                t2 = fpool.tile([16, R], FP32, tag="t2")
                nc.scalar.activation(t2[:], r2[:], AF.Exp, scale=-1.0)
                elu2 = fpool.tile([16, R], FP32, tag="elu2")
                nc.vector.scalar_tensor_tensor(
                    elu2[:], t2[:], -1.0, pre2[:], op0=OP.add, op1=OP.max
                )
                # transpose to natural [i, o2] then log_softmax over free dim
                for it in range(R // 128):
                    pn = pfp2.tile([128, 16], FP32, tag="pn")
                    nc.tensor.transpose(
                        pn[:], elu2[:, it * 128:(it + 1) * 128], idf_sb[:]
                    )
                    nmx = fpool.tile([128, 1], FP32, tag="nmx")
                    nc.vector.tensor_reduce(
                        nmx[:], pn[:], AX.X, OP.max, negate=True
                    )
                    ex = fpool.tile([128, 16], FP32, tag="ex")
                    s = fpool.tile([128, 1], FP32, tag="s")
                    nc.scalar.activation(
                        ex[:], pn[:], AF.Exp, bias=nmx[:, 0:1], accum_out=s[:, 0:1]
                    )
                    lg = fpool.tile([128, 1], FP32, tag="lg")
                    nc.scalar.activation(lg[:], s[:], AF.Ln)
                    fin = fpool.tile([128, 16], FP32, tag="fin")
                    nc.vector.tensor_scalar(
                        fin[:], pn[:], nmx[:, 0:1], lg[:, 0:1],
                        op0=OP.add, op1=OP.subtract,
                    )
                    nc.sync.dma_start(out_d[it * 128:(it + 1) * 128, :], fin[:])

    nc.compile()
    return nc


def _get_nc():
    if "nc" not in _BUILD_CACHE:
        _BUILD_CACHE["nc"] = _build_nc()
    return _BUILD_CACHE["nc"]


def _prep_inputs(x, adj, W1, a_src1, a_dst1, W2, a_src2, a_dst2):
    bf16 = ml_dtypes.bfloat16
    f32 = np.float32
    x = np.asarray(x, f32)
    adj = np.asarray(adj, f32)
    W1 = np.asarray(W1, f32)
    W2 = np.asarray(W2, f32)
    a_src1 = np.asarray(a_src1, f32)
    a_dst1 = np.asarray(a_dst1, f32)
    a_src2 = np.asarray(a_src2, f32)
    a_dst2 = np.asarray(a_dst2, f32)

    W1f = np.ascontiguousarray(W1.reshape(F_IN, F1))
    # folded score vectors: f_src[h] = x @ (W1[:,h,:] @ a_src1[h])
    wsrc = np.stack([W1[:, h, :] @ a_src1[h] for h in range(H1)], axis=1)
    wdst = np.stack([W1[:, h, :] @ a_dst1[h] for h in range(H1)], axis=1)
    W2f = np.ascontiguousarray(W2.reshape(F1, D2))
    W2a = np.zeros((F1, 18), f32)
    W2a[:, :D2] = W2f
    W2a[:, 16] = W2f @ a_src2[0]
    W2a[:, 17] = W2f @ a_dst2[0]

    xT = np.ascontiguousarray(x.T)
    ident = np.eye(128, dtype=f32)

    shared = {
        "xT": xT.astype(bf16),
        "W1f": W1f.astype(bf16),
        "wdn": wdst.astype(bf16),
        "wsn": wsrc.astype(bf16),
        "onesb": np.ones((128, 128), bf16),
        "idb": ident.astype(bf16),
        "idf16": np.eye(16, dtype=f32),
        "W2a": W2a.astype(bf16),
    }
    in_maps = []
    for c in range(NCORES):
        blkslice = slice(c * R, (c + 1) * R)
        m = dict(shared)
        m["adjT"] = np.ascontiguousarray(adj[blkslice, :].T).astype(bf16)
        m["xTo"] = np.ascontiguousarray(x[blkslice, :].T).astype(bf16)
        in_maps.append(m)
    return in_maps


def kernel(x, adj, W1, a_src1, a_dst1, W2, a_src2, a_dst2, _trace=False):
    from concourse.bass_utils import run_bass_kernel_spmd

    nc = _get_nc()
    in_maps = _prep_inputs(x, adj, W1, a_src1, a_dst1, W2, a_src2, a_dst2)
    res = run_bass_kernel_spmd(nc, in_maps, list(range(NCORES)), trace=_trace)
    out = np.concatenate(
        [np.asarray(res.results[c]["out"]) for c in range(NCORES)], axis=0
    )
    kernel.last_results = res
    return out.astype(np.float32)


# revision 29
# speedup vs baseline: 1.7564x; 1.0622x over previous
"""Two-layer GAT (8-head + 1-head) Trainium2 Bass kernel, 8-way node-sharded.

Strategy (per core c, owning row block I_c of R = N/8 nodes), layer 1:
  * Softmax over neighbors j is invariant to per-row (per-i) scaling, so
    P[j, i] ~ adjT[j, i] * max(exp((1-a)fs_i + fd_j), exp(a * fd_j))
    using exp(leakyrelu(z)) = max(exp(z), exp(a z)) and dropping the
    exp(a fs_i) row factor.  exp is evaluated by ScalarE directly as
    t1 = Exp(0.8 * fsb + fd_j) (fsb = broadcast f_src rows, fd as the
    per-partition bias); the N x R attention field then needs a SINGLE
    fused DVE op per tile: p = (t1 max vd_j) * adjT.
  * Scores live in TRANSPOSED layout [j (partition), i (free)] so both
    the aggregation out^T[o, i] = sum_j h[j, o] P[j, i] and the softmax
    denominator Z contract over the partition dim.  Z rides an all-ones
    stationary matmul, landing broadcast across all 128 partitions so
    normalization needs no further broadcast.
  * h = x @ W1 for all nodes is computed locally (replicated), with the
    fd columns falling out of the same pass via a folded [512, 8]
    matmul.
  * Layer-2 inputs h2 = h1 @ W2 (+ f_src2/f_dst2 via folded W2 columns)
    are tiny ([N, 18]); each core computes its own block and an
    AllGather distributes them; layer 2 repeats the same scheme with a
    single head.
"""

import sys

sys.path.insert(0, "/opt/trn_rl_repo")

import numpy as np
import ml_dtypes

N = 4096
F_IN = 512
H1 = 8
D1 = 128
F1 = 1024          # H1 * D1
D2 = 16
NCORES = 8
R = N // NCORES    # rows (nodes) per core
NCH = N // 128     # j-chunks of 128
NFC = F_IN // 128  # f chunks
ALPHA = 0.2
BIG = 1e38         # mask scale: adjT ships as {0, BIG}; mask = min(u, adjT)
WIDE_MM = False    # single matmul spanning 2 PSUM banks fails neuronx codegen

_BUILD_CACHE = {}


def _build_nc():
    import concourse.bacc as bacc
    import concourse.tile as tile
    import concourse.mybir as mybir

    FP32 = mybir.dt.float32
    BF16 = mybir.dt.bfloat16
    AF = mybir.ActivationFunctionType
    OP = mybir.AluOpType
    AX = mybir.AxisListType

    nc = bacc.Bacc(num_devices=NCORES)

    # ---- I/O -------------------------------------------------------------
    xT_d = nc.dram_tensor("xT", [F_IN, N], BF16, kind="ExternalInput")
    xTo_d = nc.dram_tensor("xTo", [F_IN, R], BF16, kind="ExternalInput")
    W1_d = nc.dram_tensor("W1f", [F_IN, F1], BF16, kind="ExternalInput")
    adr_d = nc.dram_tensor("adstrow", [1, F1], BF16, kind="ExternalInput")
    wsn_d = nc.dram_tensor("wsn", [F_IN, 8], BF16, kind="ExternalInput")
    adjT_d = nc.dram_tensor("adjT", [N, R], BF16, kind="ExternalInput")
    onesb_d = nc.dram_tensor("onesb", [128, 128], BF16, kind="ExternalInput")
    idb_d = nc.dram_tensor("idb", [128, 128], BF16, kind="ExternalInput")
    idf_d = nc.dram_tensor("idf16", [16, 16], FP32, kind="ExternalInput")
    W2a_d = nc.dram_tensor("W2a", [F1, 18], BF16, kind="ExternalInput")
    out_d = nc.dram_tensor("out", [R, D2], FP32, kind="ExternalOutput")

    with tile.TileContext(nc) as tc:
        with (
            tc.tile_pool(name="const", bufs=1) as cpool,
            tc.tile_pool(name="dram", bufs=1, space="DRAM") as dpool,
        ):
            # ---- resident SBUF tensors ----------------------------------
            W1_sb = cpool.tile([128, NFC * F1], BF16, tag="W1")
            adr_sb = cpool.tile([1, F1], BF16, tag="adr")
            adb_sb = cpool.tile([128, F1], BF16, tag="adb")
            wsn_sb = cpool.tile([128, NFC * 8], BF16, tag="wsn")
            xo_sb = cpool.tile([128, NFC * R], BF16, tag="xo")
            onesb_sb = cpool.tile([128, 128], BF16, tag="onesb")
            idb_sb = cpool.tile([128, 128], BF16, tag="idb")
            idf_sb = cpool.tile([16, 16], FP32, tag="idf")
            W2a_sb = cpool.tile([128, (F1 // 128) * 18], BF16, tag="W2a")
            adjT_sb = cpool.tile([128, NCH * R], BF16, tag="adjT")

            h_sb = cpool.tile([128, NCH * F1], BF16, tag="h")
            fsb = cpool.tile([128, H1 * R], BF16, tag="fsb")
            abc = cpool.tile([128, H1 * R], BF16, tag="abc")
            fsx = cpool.tile([128, 2 * R], BF16, tag="fsx")
            fd_sb = cpool.tile([128, NCH * 8], BF16, tag="fd")
            efd_sb = cpool.tile([128, NCH * 8], FP32, tag="efd")
            vd_sb = cpool.tile([128, NCH * 8], FP32, tag="vd")
            h1T = cpool.tile([128, H1 * R], BF16, tag="h1T")
            h2all_sb = cpool.tile([128, NCH * 18], BF16, tag="h2all")
            fs2row = cpool.tile([1, R], FP32, tag="fs2row")
            fsb2 = cpool.tile([128, R], BF16, tag="fsb2")
            fd2_sb = cpool.tile([128, NCH], FP32, tag="fd2")
            vd2_sb = cpool.tile([128, NCH], FP32, tag="vd2")

            h2loc = dpool.tile([R, 18], BF16, tag="h2loc")
            h2all_d = dpool.tile([N, 18], BF16, tag="h2all", addr_space="Shared")

            # ---- input DMAs, ordered so phase B can start ASAP ----------
            nc.sync.dma_start(
                W1_sb[:].rearrange("p (c o) -> p c o", o=F1),
                W1_d[:].rearrange("(c p) o -> p c o", p=128),
            )
            nc.sync.dma_start(adr_sb[:], adr_d[:])
            nc.sync.dma_start(onesb_sb[:], onesb_d[:])

            # =============================================================
            # Phase B: h = x @ W1 (all nodes) + fd columns; streams xT in
            # =============================================================
            with tc.tile_pool(name="xres", bufs=1) as xrp:
                x_sb = xrp.tile([128, NFC * N], BF16, tag="x")
                # quarter-column DMAs, q-major so early jt chunks land first
                x3 = x_sb[:].rearrange("p (c n) -> p c n", n=N)
                for q in range(4):
                    nc.sync.dma_start(
                        x3[:, :, q * 1024:(q + 1) * 1024],
                        xT_d[:, q * 1024:(q + 1) * 1024]
                        .rearrange("(c p) o -> p c o", p=128),
                    )
                # the rest of the inputs, roughly in order of first use
                nc.sync.dma_start(
                    xo_sb[:].rearrange("p (c o) -> p c o", o=R),
                    xTo_d[:].rearrange("(c p) o -> p c o", p=128),
                )
                nc.sync.dma_start(
                    wsn_sb[:].rearrange("p (c o) -> p c o", o=8),
                    wsn_d[:].rearrange("(c p) o -> p c o", p=128),
                )
                nc.sync.dma_start(
                    adjT_sb[:].rearrange("p (c o) -> p c o", o=R),
                    adjT_d[:].rearrange("(c p) o -> p c o", p=128),
                )
                nc.sync.dma_start(idb_sb[:], idb_d[:])
                nc.sync.dma_start(idf_sb[:], idf_d[:])
                nc.sync.dma_start(
                    W2a_sb[:].rearrange("p (c o) -> p c o", o=18),
                    W2a_d[:].rearrange("(c p) o -> p c o", p=128),
                )

                # broadcast a_dst row across partitions (for fd-from-h)
                with tc.tile_pool(name="padb", bufs=1, space="PSUM") as padp:
                    pad = padp.tile([128, F1], FP32, tag="pad")
                    for half in range(2):
                        nc.tensor.matmul(
                            pad[:, half * 512:(half + 1) * 512],
                            onesb_sb[0:1, :],
                            adr_sb[0:1, half * 512:(half + 1) * 512],
                            start=True, stop=True,
                        )
                    nc.scalar.activation(adb_sb[:], pad[:], AF.Copy)

                with (
                    tc.tile_pool(name="ph", bufs=3, space="PSUM") as php,
                    tc.tile_pool(name="hp", bufs=3) as hpp,
                ):
                    for jt in range(NCH):
                        ph = php.tile([128, F1], FP32, tag="ph")
                        for fc in range(NFC):
                            lhs = x_sb[:, fc * N + jt * 128:fc * N + (jt + 1) * 128]
                            st, sp = fc == 0, fc == NFC - 1
                            if WIDE_MM:
                                nc.tensor.matmul(
                                    ph[:], lhs,
                                    W1_sb[:, fc * F1:(fc + 1) * F1],
                                    start=st, stop=sp,
                                )
                            else:
                                nc.tensor.matmul(
                                    ph[:, 0:512], lhs,
                                    W1_sb[:, fc * F1:fc * F1 + 512],
                                    start=st, stop=sp,
                                )
                                nc.tensor.matmul(
                                    ph[:, 512:F1], lhs,
                                    W1_sb[:, fc * F1 + 512:(fc + 1) * F1],
                                    start=st, stop=sp,
                                )
                        nc.scalar.activation(
                            h_sb[:, jt * F1:jt * F1 + 512], ph[:, 0:512], AF.Copy
                        )
                        nc.scalar.activation(
                            h_sb[:, jt * F1 + 512:(jt + 1) * F1], ph[:, 512:F1],
                            AF.Copy,
                        )
                        # fd[j, g] = sum_o h[j, g, o] * a_dst[g, o]
                        prod = hpp.tile([128, F1], BF16, tag="prod")
                        nc.vector.tensor_mul(
                            prod[:], h_sb[:, jt * F1:(jt + 1) * F1], adb_sb[:]
                        )
                        with nc.allow_low_precision("fd accumulated in bf16"):
                            nc.vector.tensor_reduce(
                                fd_sb[:, jt * 8:(jt + 1) * 8],
                                prod[:].rearrange("p (g o) -> p g o", o=D1),
                                AX.X, OP.add,
                            )
                        nc.scalar.activation(
                            vd_sb[:, jt * 8:(jt + 1) * 8],
                            fd_sb[:, jt * 8:(jt + 1) * 8], AF.Exp,
                            scale=ALPHA,
                        )
                        nc.scalar.activation(
                            efd_sb[:, jt * 8:(jt + 1) * 8],
                            fd_sb[:, jt * 8:(jt + 1) * 8], AF.Exp,
                        )

            # =============================================================
            # Phase A: own-block f_src -> broadcast rows fsb
            # =============================================================
            with (
                tc.tile_pool(name="pfs", bufs=1, space="PSUM") as pfsp,
                tc.tile_pool(name="pab", bufs=2, space="PSUM") as pabp,
                tc.tile_pool(name="a8p", bufs=1) as a8p,
            ):
                fsT8 = pfsp.tile([8, R], FP32, tag="fs8")
                for fc in range(NFC):
                    nc.tensor.matmul(
                        fsT8[:],
                        wsn_sb[:, fc * 8:(fc + 1) * 8],
                        xo_sb[:, fc * R:(fc + 1) * R],
                        start=fc == 0, stop=fc == NFC - 1,
                    )
                fs8 = a8p.tile([8, R], BF16, tag="fs8s")
                nc.scalar.activation(fs8[:], fsT8[:], AF.Copy)
                for g in range(H1):
                    q, hf = g % 4, g // 4
                    nc.sync.dma_start(
                        fsx[32 * q:32 * q + 1, hf * R:(hf + 1) * R],
                        fs8[g:g + 1, :],
                    )
                for g in range(H1):
                    q, hf = g % 4, g // 4
                    pb = pabp.tile([128, R], FP32, tag="pab")
                    nc.tensor.matmul(
                        pb[:],
                        onesb_sb[32 * q:32 * q + 1, :],
                        fsx[32 * q:32 * q + 1, hf * R:(hf + 1) * R],
                        start=True, stop=True, tile_position=(32 * q, 0),
                    )
                    nc.scalar.activation(
                        fsb[:, g * R:(g + 1) * R], pb[:], AF.Copy
                    )
                    nc.scalar.activation(
                        abc[:, g * R:(g + 1) * R], pb[:], AF.Exp,
                        scale=1.0 - ALPHA,
                    )

            # =============================================================
            # Phase C: layer-1 attention, 2 heads (one pair) per pass
            # =============================================================
            with (
                tc.tile_pool(name="acc", bufs=2, space="PSUM") as acc,
                tc.tile_pool(name="sc", bufs=4) as spool,
                tc.tile_pool(name="nrm", bufs=2) as npool,
            ):
                for p in range(4):
                    g0, g1 = 2 * p, 2 * p + 1
                    po0 = acc.tile([128, R], FP32, tag="po0", name="po0")
                    po1 = acc.tile([128, R], FP32, tag="po1", name="po1")
                    pz = acc.tile([128, 2 * R], FP32, tag="pz", name="pz")
                    for c in range(NCH):
                        t1 = spool.tile([128, 2 * R], BF16, tag="t1")
                        up = spool.tile([128, 2 * R], BF16, tag="up")
                        pp = spool.tile([128, 2 * R], BF16, tag="pp")
                        for k, g in ((0, g0), (1, g1)):
                            if (2 * c + k) % 3 == 0:
                                # ACT path: exp(0.8 fs + fd) then max on DVE
                                nc.scalar.activation(
                                    t1[:, k * R:(k + 1) * R],
                                    fsb[:, g * R:(g + 1) * R],
                                    AF.Exp,
                                    bias=fd_sb[:, c * 8 + g:c * 8 + g + 1],
                                    scale=1.0 - ALPHA,
                                )
                                nc.vector.tensor_scalar_max(
                                    up[:, k * R:(k + 1) * R],
                                    t1[:, k * R:(k + 1) * R],
                                    vd_sb[:, c * 8 + g:c * 8 + g + 1],
                                )
                            else:
                                # DVE path: max(exp(0.8 fs) * exp(fd), vd)
                                nc.vector.tensor_scalar(
                                    up[:, k * R:(k + 1) * R],
                                    abc[:, g * R:(g + 1) * R],
                                    efd_sb[:, c * 8 + g:c * 8 + g + 1],
                                    vd_sb[:, c * 8 + g:c * 8 + g + 1],
                                    op0=OP.mult, op1=OP.max,
                                )
                        nc.vector.tensor_mul(
                            pp[:].rearrange("p (k f) -> p k f", f=R),
                            up[:].rearrange("p (k f) -> p k f", f=R),
                            adjT_sb[:, c * R:(c + 1) * R]
                            .unsqueeze(1)
                            .broadcast_to([128, 2, R]),
                        )
                        nc.tensor.matmul(
                            po0[:],
                            h_sb[:, c * F1 + g0 * D1:c * F1 + (g0 + 1) * D1],
                            pp[:, 0:R],
                            start=c == 0, stop=c == NCH - 1,
                        )
                        nc.tensor.matmul(
                            po1[:],
                            h_sb[:, c * F1 + g1 * D1:c * F1 + (g1 + 1) * D1],
                            pp[:, R:2 * R],
                            start=c == 0, stop=c == NCH - 1,
                        )
                        if WIDE_MM:
                            nc.tensor.matmul(
                                pz[:], onesb_sb[:], pp[:],
                                start=c == 0, stop=c == NCH - 1,
                            )
                        else:
                            nc.tensor.matmul(
                                pz[:, 0:R], onesb_sb[:], pp[:, 0:R],
                                start=c == 0, stop=c == NCH - 1,
                            )
                            nc.tensor.matmul(
                                pz[:, R:2 * R], onesb_sb[:], pp[:, R:2 * R],
                                start=c == 0, stop=c == NCH - 1,
                            )
                    # normalize + ELU -> h1^T (bf16)
                    zr = npool.tile([128, 2 * R], FP32, tag="zr")
                    nc.vector.reciprocal_approx_fast(zr[:], pz[:])
                    for k, g, po in ((0, g0, po0), (1, g1, po1)):
                        pre = npool.tile([128, R], FP32, tag="pre")
                        nc.vector.tensor_mul(
                            pre[:], po[:], zr[:, k * R:(k + 1) * R]
                        )
                        r = npool.tile([128, R], FP32, tag="r")
                        nc.scalar.activation(r[:], pre[:], AF.Relu, scale=-1.0)
                        t = npool.tile([128, R], FP32, tag="t")
                        nc.scalar.activation(t[:], r[:], AF.Exp, scale=-1.0)
                        nc.vector.scalar_tensor_tensor(
                            h1T[:, g * R:(g + 1) * R], t[:], -1.0, pre[:],
                            op0=OP.add, op1=OP.max,
                        )

            # =============================================================
            # Phase D: layer-2 transform + AllGather of [h2 | fs2 | fd2]
            # =============================================================
            with (
                tc.tile_pool(name="p2", bufs=2, space="PSUM") as p2p,
                tc.tile_pool(name="p2t", bufs=2, space="PSUM") as p2tp,
                tc.tile_pool(name="h2s", bufs=2) as h2p,
            ):
                for jt2 in range(R // 128):
                    ph2 = p2p.tile([128, 18], FP32, tag="ph2")
                    for g in range(H1):
                        nc.tensor.matmul(
                            ph2[:],
                            h1T[:, g * R + jt2 * 128:g * R + (jt2 + 1) * 128],
                            W2a_sb[:, g * 18:(g + 1) * 18],
                            start=g == 0, stop=g == H1 - 1,
                        )
                    h2t = h2p.tile([128, 18], BF16, tag="h2t")
                    nc.vector.tensor_copy(h2t[:], ph2[:])
                    nc.sync.dma_start(
                        h2loc[jt2 * 128:(jt2 + 1) * 128, :], h2t[:]
                    )
                    ps2 = p2tp.tile([1, 128], BF16, tag="ps2")
                    nc.tensor.transpose(ps2[:], h2t[:, 16:17], idb_sb[:])
                    nc.vector.tensor_copy(
                        fs2row[0:1, jt2 * 128:(jt2 + 1) * 128], ps2[:]
                    )

                nc.gpsimd.collective_compute(
                    "AllGather",
                    OP.bypass,
                    replica_groups=[list(range(NCORES))],
                    ins=[h2loc[:].opt()],
                    outs=[h2all_d[:].opt()],
                )
                nc.sync.dma_start(
                    h2all_sb[:].rearrange("p (c o) -> p c o", o=18),
                    h2all_d[:].rearrange("(c p) o -> p c o", p=128),
                )
                # layer-2 score factors
                nc.vector.tensor_copy(
                    fd2_sb[:, 0:NCH], h2all_sb[:, 17:NCH * 18:18]
                )
                nc.scalar.activation(
                    vd2_sb[:, 0:NCH], h2all_sb[:, 17:NCH * 18:18], AF.Exp,
                    scale=ALPHA,
                )
                a2row = h2p.tile([1, R], BF16, tag="a2row")
                nc.scalar.activation(a2row[:], fs2row[:], AF.Copy)
                pab2 = p2tp.tile([128, R], FP32, tag="pab2")
                nc.tensor.matmul(
                    pab2[:], onesb_sb[0:1, :], a2row[0:1, :],
                    start=True, stop=True,
                )
                nc.scalar.activation(fsb2[:], pab2[:], AF.Copy)

            # =============================================================
            # Phase E: layer-2 attention + ELU + log_softmax
            # =============================================================
            with (
                tc.tile_pool(name="acc2", bufs=1, space="PSUM") as acc2,
                tc.tile_pool(name="sc2", bufs=3) as spool2,
                tc.tile_pool(name="fin", bufs=2) as fpool,
                tc.tile_pool(name="pfin", bufs=2, space="PSUM") as pfp2,
            ):
                po2 = acc2.tile([16, R], FP32, tag="o2")
                pz2 = acc2.tile([128, R], FP32, tag="z2")
                for c in range(NCH):
                    t12 = spool2.tile([128, R], BF16, tag="t12")
                    nc.scalar.activation(
                        t12[:], fsb2[:], AF.Exp,
                        bias=fd2_sb[:, c:c + 1], scale=1.0 - ALPHA,
                    )
                    u2 = spool2.tile([128, R], BF16, tag="u2")
                    nc.vector.tensor_scalar_max(
                        u2[:], t12[:], vd2_sb[:, c:c + 1]
                    )
                    p2t = spool2.tile([128, R], BF16, tag="p2t")
                    nc.vector.tensor_mul(
                        p2t[:], u2[:], adjT_sb[:, c * R:(c + 1) * R]
                    )
                    nc.tensor.matmul(
                        po2[:], h2all_sb[:, c * 18:c * 18 + 16], p2t[:],
                        start=c == 0, stop=c == NCH - 1,
                    )
                    nc.tensor.matmul(
                        pz2[:], onesb_sb[:], p2t[:],
                        start=c == 0, stop=c == NCH - 1,
                    )
                zr2 = fpool.tile([16, R], FP32, tag="zr2")
                nc.vector.reciprocal_approx_fast(zr2[:], pz2[0:16, :])
                pre2 = fpool.tile([16, R], FP32, tag="pre2")
                nc.vector.tensor_mul(pre2[:], po2[:], zr2[:])
                r2 = fpool.tile([16, R], FP32, tag="r2")
                nc.scalar.activation(r2[:], pre2[:], AF.Relu, scale=-1.0)
                t2 = fpool.tile([16, R], FP32, tag="t2")
                nc.scalar.activation(t2[:], r2[:], AF.Exp, scale=-1.0)
                elu2 = fpool.tile([16, R], FP32, tag="elu2")
                nc.vector.scalar_tensor_tensor(
                    elu2[:], t2[:], -1.0, pre2[:], op0=OP.add, op1=OP.max
                )
                # transpose to natural [i, o2] then log_softmax over free dim
                for it in range(R // 128):
                    pn = pfp2.tile([128, 16], FP32, tag="pn")
                    nc.tensor.transpose(
                        pn[:], elu2[:, it * 128:(it + 1) * 128], idf_sb[:]
                    )
                    nmx = fpool.tile([128, 1], FP32, tag="nmx")
                    nc.vector.tensor_reduce(
                        nmx[:], pn[:], AX.X, OP.max, negate=True
                    )
                    ex = fpool.tile([128, 16], FP32, tag="ex")
                    s = fpool.tile([128, 1], FP32, tag="s")
                    nc.scalar.activation(
                        ex[:], pn[:], AF.Exp, bias=nmx[:, 0:1], accum_out=s[:, 0:1]
                    )
                    lg = fpool.tile([128, 1], FP32, tag="lg")
                    nc.scalar.activation(lg[:], s[:], AF.Ln)
                    fin = fpool.tile([128, 16], FP32, tag="fin")
                    nc.vector.tensor_scalar(
                        fin[:], pn[:], nmx[:, 0:1], lg[:, 0:1],
                        op0=OP.add, op1=OP.subtract,
                    )
                    nc.sync.dma_start(out_d[it * 128:(it + 1) * 128, :], fin[:])

    nc.compile()
    return nc


def _get_nc():
    if "nc" not in _BUILD_CACHE:
        _BUILD_CACHE["nc"] = _build_nc()
    return _BUILD_CACHE["nc"]


def _prep_inputs(x, adj, W1, a_src1, a_dst1, W2, a_src2, a_dst2):
    bf16 = ml_dtypes.bfloat16
    f32 = np.float32
    x = np.asarray(x, f32)
    adj = np.asarray(adj, f32)
    W1 = np.asarray(W1, f32)
    W2 = np.asarray(W2, f32)
    a_src1 = np.asarray(a_src1, f32)
    a_dst1 = np.asarray(a_dst1, f32)
    a_src2 = np.asarray(a_src2, f32)
    a_dst2 = np.asarray(a_dst2, f32)

    W1f = np.ascontiguousarray(W1.reshape(F_IN, F1))
    # folded score vectors: f_src[h] = x @ (W1[:,h,:] @ a_src1[h])
    wsrc = np.stack([W1[:, h, :] @ a_src1[h] for h in range(H1)], axis=1)
    wdst = np.stack([W1[:, h, :] @ a_dst1[h] for h in range(H1)], axis=1)
    W2f = np.ascontiguousarray(W2.reshape(F1, D2))
    W2a = np.zeros((F1, 18), f32)
    W2a[:, :D2] = W2f
    W2a[:, 16] = W2f @ a_src2[0]
    W2a[:, 17] = W2f @ a_dst2[0]

    xT = np.ascontiguousarray(x.T)
    ident = np.eye(128, dtype=f32)

    shared = {
        "xT": xT.astype(bf16),
        "W1f": W1f.astype(bf16),
        "adstrow": np.ascontiguousarray(a_dst1.reshape(1, F1)).astype(bf16),
        "wsn": wsrc.astype(bf16),
        "onesb": np.ones((128, 128), bf16),
        "idb": ident.astype(bf16),
        "idf16": np.eye(16, dtype=f32),
        "W2a": W2a.astype(bf16),
    }
    in_maps = []
    for c in range(NCORES):
        blkslice = slice(c * R, (c + 1) * R)
        m = dict(shared)
        m["adjT"] = np.ascontiguousarray(adj[blkslice, :].T).astype(bf16)
        m["xTo"] = np.ascontiguousarray(x[blkslice, :].T).astype(bf16)
        in_maps.append(m)
    return in_maps


def kernel(x, adj, W1, a_src1, a_dst1, W2, a_src2, a_dst2, _trace=False):
    from concourse.bass_utils import run_bass_kernel_spmd

    nc = _get_nc()
    in_maps = _prep_inputs(x, adj, W1, a_src1, a_dst1, W2, a_src2, a_dst2)
    res = run_bass_kernel_spmd(nc, in_maps, list(range(NCORES)), trace=_trace)
    out = np.concatenate(
        [np.asarray(res.results[c]["out"]) for c in range(NCORES)], axis=0
    )
    kernel.last_results = res
    return out.astype(np.float32)
